# revision 22
# baseline (speedup 1.0000x reference)
"""Block-diagonal 2x2 equalizer kernel for Trainium2 (8 NeuronCores).

Per point (b, u, s, f) solves the 2x2 system M x = v by Cramer's rule:
    m_ij = h[b, pi[u], i, 0, 2u+j, s, f]   (only 1/4 of h is needed)
    det  = m00*m11 - m01*m10
    x0   = (m11*v0 - m01*v1) / det
    x1   = (m00*v1 - m10*v0) / det
    out[b, u, a, s, f] = x_a

Sharding: data-parallel over batch, 2 batches per core on 8 cores. The host
gathers (precoding_ind) and packs operand planes into one [128, 5, 1792]
f32 HBM tensor per core: planes 0-3 = m00, m01, m11, m10 (f32 - the det
cancellation at near-singular blocks needs full input precision; fp16 h
fails at rel 0.6), plane 4 = the fp16 bits of [v0|v1] viewed as f32 words
(device reads them back via AP bitcast).  y/q/r/rdet/x run in fp16
(validated: rel err 3.3e-4 vs the 2e-2 gate) which halves their DMA bytes
and doubles DVE throughput.

DMA strategy (the v2 lesson): concurrent HWDGE DMAs round-robin at packet
granularity so ungated loads all complete together, while sem-gated serial
loads lose ~1.75us per gate to completion-receipt + re-issue latency.
Instead the input is ONE tensor sliced into 8 column strips issued on one
ring, where strip s+1 is issued after wait_ge(semL[s], 8) - half the 16
per-engine completion increments - so descriptor generation of the next
strip overlaps the tail of the previous: a gapless stream that still
completes in consumption order.  Strip widths taper up then down
(128,128,256,256,288,288,224,224) so the first compute chunk starts early
and the last loads aren't late.

Compute: 4 column chunks (256,512,576,448) of 6 DVE ops each:
    mulP   [p0|p1] = [m00|m01] * [m11|m10]      f32, 1x mode
    subDet det     = p0 - p1                     f32
    mulQA  [q0|q3] = [m11|m10] * bcast(v0)       fp16, 2x mode
    mulQB  [q2|q1] = [m00|m01] * bcast(v1)       fp16
    subR   [r0|r1] = [q0|q2] - [q1|q3]           fp16
    final  [x0|x1] = [r0|r1] * bcast(rdet)       fp16
ACT casts h f32->fp16 for the q-path (ACT is otherwise idle; the cast lets
both q muls run in DVE 2x mode), does rdet = Reciprocal(det) -> fp16 via
raw InstActivation (bass blocks the wrapper; HW spline err 2.2e-5), and
issues the 4 output stores on its own HWDGE ring.

Raw Bass (no TileContext): walrus allows one sync-wait per instruction, so
all waits are standalone wait_ge; buffers are written once per column
region (pure dataflow, no WAR hazards).
"""

from contextlib import ExitStack

import numpy as np

import concourse.bass as bass
import concourse.mybir as mybir
from concourse.bass_utils import run_bass_kernel_spmd

# Problem shapes (hardcoded per contract)
B, U, A, NTX, T, S, F = 16, 4, 2, 1, 8, 14, 2048
SF = S * F               # 28672
NCORES = 8
BPC = B // NCORES        # 2 batches per core
QW = 448                 # SF = 64 * 448
ROWS = SF // QW          # 64 rows -> partition p = b*64 + row
FW = U * QW              # 1792 full free width (u-major columns)

# column strips (DMA) and chunks (compute); strip s feeds chunk s.
# Growing widths: compute-per-col ~= stream-per-col, so DVE can start on a
# small first strip and track the stream without stalls; a small-ish last
# chunk keeps the post-stream tail short.
SW = (192, 288, 384, 448, 352, 128)
CW = (192, 288, 384, 448, 352, 128)
SOFF = [sum(SW[:i]) for i in range(len(SW) + 1)]
COFF = [sum(CW[:i]) for i in range(len(CW) + 1)]
NST = len(SW)
NCK = len(CW)

F16 = np.float16
F32 = np.float32

TRACE = False
LAST_RESULTS = None


def _pk(d, dtype):
    """[BPC, U, SF] -> [128, FW]: p = b*ROWS + sf//QW, col = u*QW + sf%QW."""
    d = d.reshape(BPC, U, ROWS, QW)
    return np.ascontiguousarray(
        d.transpose(0, 2, 1, 3).reshape(BPC * ROWS, FW)
    ).astype(dtype)


def _unpk(t):
    """Inverse of _pk: [128, FW] -> [BPC, U, SF]."""
    t = t.reshape(BPC, ROWS, U, QW).transpose(0, 2, 1, 3)
    return t.reshape(BPC, U, SF)


def _build_nc():
    f32 = mybir.dt.float32
    f16 = mybir.dt.float16
    nc = bass.Bass("TRN2")
    # planes: 0=m11 1=m10 2=m00 3=m01, 4 = fp16 [v0|v1] bits as f32 words
    hAll = nc.dram_tensor("hAll", [128, 5, FW], f32, kind="ExternalInput")
    xO = nc.dram_tensor("xO", [128, 2, FW], f16, kind="ExternalOutput")

    with ExitStack() as ctx:
        sb = ctx.enter_context
        HY = sb(nc.sbuf_tensor("HY", [128, 5, FW], f32))
        H6 = sb(nc.sbuf_tensor("H6", [128, 4, FW], f16))
        P = sb(nc.sbuf_tensor("P", [128, 2, FW], f32))
        DET = sb(nc.sbuf_tensor("DET", [128, FW], f32))
        RD = sb(nc.sbuf_tensor("RD", [128, 1, FW], f16))
        # Q plane order: 0=q0 1=q3 2=q2 3=q1 (merged mul's natural output)
        Q = sb(nc.sbuf_tensor("Q", [128, 4, FW], f16))
        R = sb(nc.sbuf_tensor("R", [128, 2, FW], f16))
        X = sb(nc.sbuf_tensor("X", [128, 2, FW], f16))
        WRM = sb(nc.sbuf_tensor("WRM", [128, 8], f32))   # act-table warmup
        semL = [sb(nc.semaphore(f"semL{s}")) for s in range(NST)]
        semO = [sb(nc.semaphore(f"semO{c}")) for c in range(NCK)]
        dve_sem = sb(nc.semaphore("dve_sem"))
        act_sem = sb(nc.semaphore("act_sem"))

        yv = HY[:, 4:5, :].bitcast(f16)  # [128, 1, 2*FW]: cols 0:FW v0, FW: v1

        with nc.Block() as block:

            @block.sync
            def _(sync):
                # ungated back-to-back issue: all loads share the qSPDynamicHW
                # ring, which executes FIFO per issuing engine - strips
                # complete in order at full rate with no inter-DMA gate gaps
                for s in range(NST):
                    c0, c1 = SOFF[s], SOFF[s + 1]
                    sync.dma_start(
                        out=HY[:, :, c0:c1], in_=hAll[:, :, c0:c1]
                    ).then_inc(semL[s], 16)

            # dve_sem: chunk c ops are 5c+1 .. 5c+5
            @block.vector
            def _(vector):
                for c in range(NCK):
                    a = 2 * c
                    c0, c1 = COFF[c], COFF[c + 1]
                    w = c1 - c0
                    vector.wait_ge(semL[c], 16)
                    # [p0|p1] = [m00|m01] * [m11|m10]
                    vector.tensor_mul(
                        P[:, :, c0:c1], HY[:, 2:4, c0:c1], HY[:, 0:2, c0:c1]
                    ).then_inc(dve_sem, 1)                                   # d+1
                    vector.tensor_sub(
                        DET[:, c0:c1], P[:, 0, c0:c1], P[:, 1, c0:c1]
                    ).then_inc(dve_sem, 1)                                   # d+2
                    vector.wait_ge(act_sem, a + 1)
                    # merged q-mul: [[q0,q3],[q2,q1]] =
                    #   [[m11,m10],[m00,m01]] * [[v0,v0],[v1,v1]]
                    # y-bits are strip-local: fp16 cols [2*c0, 2*c0+w) = v0,
                    # [2*c0+w, 2*c1) = v1 of this chunk's columns
                    vector.tensor_mul(
                        Q[:, :, c0:c1].rearrange("p (a b) w -> p a b w", a=2),
                        H6[:, :, c0:c1].rearrange("p (a b) w -> p a b w", a=2),
                        yv[:, 0, 2 * c0:2 * c1]
                        .rearrange("p (a w) -> p a w", a=2)
                        .unsqueeze(2)
                        .broadcast_to([128, 2, 2, w]),
                    ).then_inc(dve_sem, 1)                                   # d+3
                    # [r0|r1] = [q0|q2] - [q1|q3]
                    vector.tensor_sub(
                        R[:, :, c0:c1], Q[:, 0::2, c0:c1], Q[:, 3::-2, c0:c1]
                    ).then_inc(dve_sem, 1)                                   # d+4
                    vector.wait_ge(act_sem, a + 2)
                    vector.tensor_mul(
                        X[:, :, c0:c1],
                        R[:, :, c0:c1],
                        RD[:, :, c0:c1].broadcast_to([128, 2, w]),
                    ).then_inc(dve_sem, 1)                                   # d+5

            @block.scalar
            def _(scalar):
                f32dt = mybir.dt.float32

                def cast(c):
                    c0, c1 = COFF[c], COFF[c + 1]
                    scalar.wait_ge(semL[c], 16)
                    scalar.activation(
                        H6[:, :, c0:c1],
                        HY[:, 0:4, c0:c1],
                        mybir.ActivationFunctionType.Copy,
                    ).then_inc(act_sem, 1)

                def recip(c):
                    c0, c1 = COFF[c], COFF[c + 1]
                    scalar.wait_ge(dve_sem, 5 * c + 2)
                    scalar.add_instruction(
                        mybir.InstActivation(
                            name=nc.get_next_instruction_name(),
                            func=mybir.ActivationFunctionType.Reciprocal,
                            ins=[
                                scalar.lower_ap(DET[:, c0:c1]),
                                mybir.ImmediateValue(dtype=f32dt, value=0.0),
                                mybir.ImmediateValue(dtype=f32dt, value=1.0),
                                mybir.ImmediateValue(dtype=f32dt, value=0.0),
                            ],
                            outs=[scalar.lower_ap(RD[:, 0, c0:c1])],
                        )
                    ).then_inc(act_sem, 1)

                def store(c):
                    c0, c1 = COFF[c], COFF[c + 1]
                    scalar.wait_ge(dve_sem, 5 * c + 5)
                    scalar.dma_start(
                        out=xO[:, :, c0:c1], in_=X[:, :, c0:c1]
                    ).then_inc(semO[c], 16)

                # preload the activation table before any data arrives so
                # cast0 doesn't eat the ~1.3us ACT_TABLE_LOAD on the hot path
                scalar.activation(
                    WRM[:, 4:8], WRM[:, 0:4], mybir.ActivationFunctionType.Copy
                )
                # order: each cast as soon as its strip lands, never stuck
                # behind a store (stores wait on DVE finals). act_sem counts
                # must stay (cast c, recip c) = (2c+1, 2c+2) for DVE's waits,
                # so emit cast/recip in chunk order, stores as late as allowed.
                cast(0)
                recip(0)
                for c in range(1, NCK):
                    cast(c)
                    store(c - 1)
                    recip(c)
                store(NCK - 1)
                for c in range(NCK):
                    scalar.wait_ge(semO[c], 16)

    return nc


def make_in_maps(y, h, precoding_ind):
    """Host-side gather + pack. Returns per-core input maps."""
    y = np.asarray(y)
    h = np.asarray(h)
    pi = np.asarray(precoding_ind).astype(np.int64)

    hg = h[:, pi[0]]                                     # [B, U, A, NTX, T, S, F]
    # hsel[b, u, i, j] = hg[b, u, i, 0, 2u+j]  -> components c = i*2+j
    hsel = np.stack(
        [hg[:, u, :, 0, 2 * u:2 * u + 2] for u in range(U)], axis=1
    )                                                    # [B, U, A(i), 2(j), S, F]
    hsel = np.ascontiguousarray(hsel).reshape(B, U, 4, SF).astype(F32)
    yr = np.ascontiguousarray(y).reshape(B, U, A, SF).astype(F16)

    in_maps = []
    for cid in range(NCORES):
        b0 = cid * BPC
        hs = hsel[b0:b0 + BPC]                           # [BPC, U, 4, SF]
        ys = yr[b0:b0 + BPC]                             # [BPC, U, A, SF]
        hA = np.empty((128, 5, FW), F32)
        # device plane order: m11 (c3), m10 (c2), m00 (c0), m01 (c1)
        for p, c in enumerate((3, 2, 0, 1)):
            hA[:, p, :] = _pk(hs[:, :, c], F32)
        v0p = _pk(ys[:, :, 0], F16)
        v1p = _pk(ys[:, :, 1], F16)                      # [128, FW] fp16 each
        # strip-local fold: plane-4 words of strip s = [v0|v1] of its columns
        for s in range(NST):
            c0, c1 = SOFF[s], SOFF[s + 1]
            yb = np.concatenate([v0p[:, c0:c1], v1p[:, c0:c1]], axis=1)
            hA[:, 4, c0:c1] = np.ascontiguousarray(yb).view(F32)
        in_maps.append({"hAll": np.ascontiguousarray(hA)})
    return in_maps


def assemble_output(results):
    """Per-core xO [128, 2, FW] fp16 -> full [B, U, A, S, F] f32."""
    out = np.empty((B, U, A, S, F), F32)
    for c in range(NCORES):
        xo = np.asarray(results[c]["xO"]).astype(F32)
        for a in range(A):
            out[c * BPC:(c + 1) * BPC, :, a] = _unpk(xo[:, a, :]).reshape(
                BPC, U, S, F
            )
    return out


def kernel(y, h, precoding_ind):
    global LAST_RESULTS
    in_maps = make_in_maps(y, h, precoding_ind)
    nc = _build_nc()
    res = run_bass_kernel_spmd(nc, in_maps, list(range(NCORES)), trace=TRACE)
    LAST_RESULTS = res
    return assemble_output(res.results)
